# revision 1
# baseline (speedup 1.0000x reference)
"""Multi-head self-attention (B=4, S=2048, D=1024, H=16) on 8 TRN2 NeuronCores.

Sharding: batch x head-group. Core c handles batch b=c//2 and heads
[8*(c%2), 8*(c%2)+8). Each core computes QKV projection, attention and its
partial output projection; the host sums the two head-group partials per batch
and adds b_proj.

Per-core dataflow (all matmuls float32r = full PE rate, ~1.5e-4 rounding):
  stage 1: Y^T = [Q^T; K^T] = wqk^T-free matmul(lhsT=wqk, rhs=x^T) -> [1024f, 2048t]
           V   = matmul(lhsT=x^T chunk, rhs=wv)                    -> [2048t, 512f]
  stage 2: S^T[k,q] = K_h Q_h^T via row-tiled head pairs (d=64 contraction)
           P^T = exp(S^T * 0.125) on ACT (ScalarE), reading 2-bank PSUM tiles
  stage 3: C~^T = [V_h|1]^T P^T  (ones-column makes row 64 the softmax denom)
           normalize: recip(sums) -> DRAM -> partition-broadcast DMA -> DVE mul
  stage 4: out = C^T-proj: matmul(lhsT=C^T chunk, rhs=w_proj rows)  -> [2048t, 1024]
"""
import numpy as np

import concourse.bacc as bacc
import concourse.tile as tile
from concourse import bass_isa, mybir
from concourse import bass_utils

P = 128
B, S, D = 4, 2048, 1024
H_TOT, HD = 16, 64
H = 8          # heads per core
NPAIR = 4      # head pairs per core
SCALE = HD ** -0.5
DCH = D // P   # 8 contraction chunks
NTT = S // P   # 16 token tiles
f32 = mybir.dt.float32
f32r = mybir.dt.float32r
AF = mybir.ActivationFunctionType

_CACHED_NC = None


def build_nc():
    nc = bacc.Bacc(trn_type="TRN2", target_bir_lowering=False, debug=False)
    xt = nc.dram_tensor("xt", [D, S], f32r, kind="ExternalInput").ap()
    wqk = nc.dram_tensor("wqk", [D, 2 * H * HD], f32r, kind="ExternalInput").ap()
    wv = nc.dram_tensor("wv", [D, H * HD], f32r, kind="ExternalInput").ap()
    wp = nc.dram_tensor("wp", [H * HD, D], f32r, kind="ExternalInput").ap()
    bqk = nc.dram_tensor("bqk", [8, P], f32, kind="ExternalInput").ap()
    vbias = nc.dram_tensor("vbias", [P, NPAIR * 130], f32, kind="ExternalInput").ap()
    out = nc.dram_tensor("out", [S, D], f32, kind="ExternalOutput").ap()

    with tile.TileContext(nc) as tc:
        with tc.tile_pool(name="persist", bufs=1) as pp:
            # persistent SBUF tensors
            yt = [pp.tile([P, S], f32r, name=f"yt{f}") for f in range(8)]
            vp = pp.tile([P, NTT, NPAIR * 130], f32r, name="vp")
            vbias_t = pp.tile([P, NPAIR * 130], f32, name="vbias_t")

            # ---------------- stage 1: QKV projections ----------------
            with (
                tc.tile_pool(name="s1w", bufs=1) as s1w,
                tc.tile_pool(name="s1x", bufs=2) as s1x,
                tc.tile_pool(name="s1ps", bufs=4, space="PSUM") as s1ps,
            ):
                TCH = 256
                NCH = S // TCH
                # per-feature-tile weight tiles so the first matmuls start early
                wqk_f = [s1w.tile([P, DCH, P], f32r, name=f"wqkf{f}")
                         for f in range(8)]
                wv_t = s1w.tile([P, DCH, H * HD], f32r, name="wv_t")
                bqk_t = s1w.tile([P, 8], f32, name="bqk_t")
                wqk4 = wqk.rearrange("(c p) (f g) -> p c f g", p=P, f=8)
                # first xt chunks + weight tiles up front; weights go on the
                # ACT HWDGE ring so they don't queue behind the xt stream
                xt_ts = [s1x.tile([P, DCH, TCH], f32r, name="xt_t")
                         for _ in range(2)]
                nc.sync.dma_start(xt_ts[0][:],
                                  xt[:, 0:TCH].rearrange("(c p) s -> p c s", p=P))
                nc.scalar.dma_start(wqk_f[0][:], wqk4[:, :, 0, :])
                nc.scalar.dma_start(bqk_t[:], bqk.rearrange("a p -> p a"))
                nc.scalar.dma_start(vbias_t[:], vbias[:])
                for f in range(1, 8):
                    nc.scalar.dma_start(wqk_f[f][:], wqk4[:, :, f, :])
                nc.sync.dma_start(
                    xt_ts[1][:],
                    xt[:, TCH:2 * TCH].rearrange("(c p) s -> p c s", p=P))
                nc.scalar.dma_start(wv_t[:], wv.rearrange("(c p) f -> p c f", p=P))

                for t in range(NCH):  # 256-token chunks
                    tsl = slice(t * TCH, (t + 1) * TCH)
                    if t < 2:
                        xt_t = xt_ts[t]
                    else:
                        xt_t = s1x.tile([P, DCH, TCH], f32r, name="xt_t")
                        nc.sync.dma_start(
                            xt_t[:], xt[:, tsl].rearrange("(c p) s -> p c s", p=P))
                    for f in range(8):  # Q,K feature tiles
                        ps = s1ps.tile([P, TCH], f32, name="s1pq")
                        for i in range(DCH):
                            nc.tensor.matmul(
                                ps[:], wqk_f[f][:, i, :], xt_t[:, i, :],
                                start=(i == 0), stop=(i == DCH - 1))
                        nc.vector.tensor_scalar(
                            out=yt[f][:, tsl], in0=ps[:], scalar1=bqk_t[:, f:f + 1],
                            scalar2=None, op0=mybir.AluOpType.add)
                    if t == 0:
                        # vp bias+ones init, deferred so the xt/weight streams
                        # get the startup DMA bandwidth
                        for tt2 in range(NTT):
                            nc.gpsimd.dma_start(vp[:, tt2, :], vbias[:])
                    for sub in range(TCH // P):  # V for 128-token subtiles
                        tt = t * (TCH // P) + sub
                        ps = s1ps.tile([P, 512], f32, name="s1p")
                        for i in range(DCH):
                            nc.tensor.matmul(
                                ps[:], xt_t[:, i, sub * P:(sub + 1) * P], wv_t[:, i, :],
                                start=(i == 0), stop=(i == DCH - 1))
                        vpt = vp[:, tt, :].rearrange("p (j k c) -> p j k c",
                                                     j=NPAIR, k=2)
                        vb4 = vbias_t[:].rearrange("p (j k c) -> p j k c",
                                                   j=NPAIR, k=2)
                        nc.vector.tensor_tensor(
                            out=vpt[:, :, :, 0:HD],
                            in0=ps[:].rearrange("p (j k c) -> p j k c", j=NPAIR, k=2),
                            in1=vb4[:, :, :, 0:HD],
                            op=mybir.AluOpType.add)

            # ---------------- stages 2+3: attention ----------------
            # Per k-chunk: one [128,1024] PSUM tile holds S^T for both heads of
            # the pair (even in cols 0:512, odd in 512:1024), one ACT exp per
            # k-chunk, PV software-pipelined one k-chunk behind.
            ct = [pp.tile([P, S], f32r, name=f"ct{j}") for j in range(NPAIR)]
            with (
                tc.tile_pool(name="s4w", bufs=1) as s4w,
                tc.tile_pool(name="att", bufs=1) as att,
                tc.tile_pool(name="s4o", bufs=2) as s4o,
                tc.tile_pool(name="spt", bufs=2, space="PSUM") as sptp,
                tc.tile_pool(name="cps", bufs=2, space="PSUM") as cpsp,
                tc.tile_pool(name="s4ps", bufs=2, space="PSUM") as s4ps,
            ):
                wp_t = s4w.tile([P, NPAIR, D], f32r, name="wp_t")
                nc.scalar.dma_start(wp_t[:], wp.rearrange("(c p) f -> p c f", p=P))

                # zeros rows 0:63 + per-norm recip row 64; partition all-reduce
                # (add) then replicates the recip row across all partitions
                zt = att.tile([65, 1024], f32, name="zt", bufs=1)
                nc.vector.memset(zt[0:HD, :], 0.0)

                def emit_norm(j, qa, cps_e, cps_o):
                    nc.vector.reciprocal(zt[64:65, 0:512], cps_e[64:65, :])
                    nc.vector.reciprocal(zt[64:65, 512:1024], cps_o[64:65, :])
                    rbc = att.tile([65, 1024], f32, name="rbc", bufs=2)
                    nc.gpsimd.partition_all_reduce(
                        rbc[:], zt[:], channels=65,
                        reduce_op=bass_isa.ReduceOp.add)
                    nc.vector.tensor_mul(ct[j][0:HD, qa], cps_e[0:HD, :],
                                         rbc[0:HD, 0:512])
                    cttmp = att.tile([HD, 512], f32r, name="cttmp", bufs=1)
                    nc.vector.tensor_mul(cttmp[:], cps_o[0:HD, :],
                                         rbc[0:HD, 512:1024])
                    nc.sync.dma_start(ct[j][HD:P, qa], cttmp[:])

                # projection work for one token tile, emitted as a list of
                # closures so matmuls drip into the PE stream without bursts
                def proj_steps(tt):
                    tsl = slice(tt * P, (tt + 1) * P)
                    steps = []
                    state = {}

                    def mk_mm(half, fc):
                        def f():
                            if fc == 0:
                                state[half] = s4ps.tile([P, 512], f32, name="s4p")
                            nc.tensor.matmul(
                                state[half][:], ct[fc][:, tsl],
                                wp_t[:, fc, half * 512:(half + 1) * 512],
                                start=(fc == 0), stop=(fc == NPAIR - 1))
                            if fc == NPAIR - 1:
                                o_sb = s4o.tile([P, 512], f32, name="o_sb",
                                                bufs=4)
                                nc.vector.tensor_copy(o_sb[:], state[half][:])
                                nc.sync.dma_start(
                                    out[tsl, half * 512:(half + 1) * 512],
                                    o_sb[:])
                        return f

                    for half in range(2):
                        for fc in range(NPAIR):
                            steps.append(mk_mm(half, fc))
                    return steps

                norm_pending = None
                proj_queue = []
                for qc in range(4):  # 512-wide query chunks, outer
                    qa = slice(qc * 512, (qc + 1) * 512)
                    for j in range(NPAIR):
                        qt, kt = yt[j], yt[NPAIR + j]
                        cps_e = cps_o = None
                        pv_pending = None
                        for kc in range(NTT):
                            ksl = slice(kc * P, (kc + 1) * P)
                            spt = sptp.tile([P, 1024], f32, name="spt")
                            nc.tensor.matmul(spt[:, 0:512], kt[0:HD, ksl],
                                             qt[0:HD, qa], start=True, stop=True)
                            nc.tensor.matmul(spt[:, 512:1024], kt[HD:P, ksl],
                                             qt[HD:P, qa], start=True, stop=True)
                            ppt = att.tile([P, 1024], f32r, name="ppt", bufs=4)
                            nc.scalar.activation(ppt[:], spt[:], AF.Exp,
                                                 scale=SCALE)
                            if kc == 1 and norm_pending is not None:
                                # previous (qc,j) normalization, deferred past
                                # this iteration's first two S/exp to hide its
                                # recip -> all-reduce -> mul chain
                                emit_norm(*norm_pending)
                                norm_pending = None
                            if pv_pending is not None:
                                if cps_e is None:
                                    cps_e = cpsp.tile([65, 512], f32, name="cps")
                                    cps_o = cpsp.tile([65, 512], f32, name="cps")
                                _emit_pv(nc, cps_e, cps_o, vp, pv_pending[0],
                                         pv_pending[1], j)
                            pv_pending = (kc, ppt)
                            if proj_queue and kc % 2 == 1:
                                proj_queue.pop(0)()  # drip one projection step
                        _emit_pv(nc, cps_e, cps_o, vp, pv_pending[0],
                                 pv_pending[1], j)
                        norm_pending = (j, qa, cps_e, cps_o)
                    # queue projection for this query chunk's 4 token tiles
                    # (runnable once this qc's last norm flushes next sweep)
                    for tt in range(qc * 4, (qc + 1) * 4):
                        proj_queue.extend(proj_steps(tt))
                emit_norm(*norm_pending)
                for step in proj_queue:
                    step()

    nc.finalize()
    return nc


def _emit_pv(nc, cps_e, cps_o, vp, kc, ppt, j):
    nc.tensor.matmul(cps_e[0:65, :], vp[:, kc, j * 130:j * 130 + 65],
                     ppt[:, 0:512], start=(kc == 0), stop=(kc == NTT - 1))
    nc.tensor.matmul(cps_o[0:65, :], vp[:, kc, j * 130 + 65:j * 130 + 130],
                     ppt[:, 512:1024], start=(kc == 0), stop=(kc == NTT - 1))


def get_nc():
    global _CACHED_NC
    if _CACHED_NC is None:
        _CACHED_NC = build_nc()
    return _CACHED_NC


def make_in_maps(x, w_qkv, b_qkv, w_proj):
    """Host-side sharding: one input dict per core."""
    w = np.asarray(w_qkv, np.float32).reshape(D, 3, H_TOT, HD)
    bq3 = np.asarray(b_qkv, np.float32).reshape(3, H_TOT, HD)
    in_maps = []
    for c in range(8):
        b, hg = c // 2, c % 2
        hs = slice(hg * H, (hg + 1) * H)
        wqk_c = np.ascontiguousarray(
            np.concatenate([w[:, 0, hs, :].reshape(D, H * HD),
                            w[:, 1, hs, :].reshape(D, H * HD)], axis=1))
        wv_c = np.ascontiguousarray(w[:, 2, hs, :].reshape(D, H * HD))
        wp_c = np.ascontiguousarray(
            np.asarray(w_proj, np.float32).reshape(H_TOT, HD, D)[hs].reshape(H * HD, D))
        bqk_c = np.ascontiguousarray(
            np.concatenate([bq3[0, hs].reshape(H * HD),
                            bq3[1, hs].reshape(H * HD)]).reshape(8, P))
        bv = bq3[2, hs].reshape(H * HD)
        vbias_c = np.zeros((P, NPAIR * 130), np.float32)
        for j in range(NPAIR):
            vbias_c[:, j * 130:j * 130 + HD] = bv[(2 * j) * HD:(2 * j + 1) * HD]
            vbias_c[:, j * 130 + HD] = 1.0
            vbias_c[:, j * 130 + 65:j * 130 + 65 + HD] = \
                bv[(2 * j + 1) * HD:(2 * j + 2) * HD]
            vbias_c[:, j * 130 + 129] = 1.0
        xt_c = np.ascontiguousarray(np.asarray(x[b], np.float32).T)
        in_maps.append({"xt": xt_c, "wqk": wqk_c, "wv": wv_c, "wp": wp_c,
                        "bqk": bqk_c, "vbias": vbias_c})
    return in_maps


def assemble(results, b_proj):
    out = np.empty((B, S, D), np.float32)
    bp = np.asarray(b_proj, np.float32)
    for b in range(B):
        out[b] = results[2 * b]["out"] + results[2 * b + 1]["out"] + bp
    return out


def kernel(x, w_qkv, b_qkv, w_proj, b_proj):
    nc = get_nc()
    in_maps = make_in_maps(x, w_qkv, b_qkv, w_proj)
    res = bass_utils.run_bass_kernel_spmd(nc, in_maps, core_ids=list(range(8)),
                                          trace=False)
    return assemble(res.results, b_proj)



# revision 6
# speedup vs baseline: 4.1787x; 4.1787x over previous
"""Multi-head self-attention (B=4, S=2048, D=1024, H=16) on 8 TRN2 NeuronCores.

Host I/O is the bottleneck for this problem (per-call staging of declared
I/O buffers costs ~78us/MB in + ~94us/MB out, dwarfing the ~0.5ms compute),
so the design minimizes declared bytes: every input byte is declared on
exactly one core, in bf16, and redistributed on-device with collectives.

Sharding: batch x head-group compute (core c: batch b=c//2, heads
[8*(c%2), 8*(c%2)+8)), but disjoint I/O shards:
  xh  [D, 1024] bf16 -- x[b]^T token-half c%2 (pair-AllGather -> full xt)
  wsh [D, 512]  bf16 -- quarter of this head-group's packed weights
                        (4-core AllGather over {c%2, c%2+2, ...} -> full)
  oh  [1024, D] bf16 -- ReduceScatter(add) over the batch pair sums the two
                        head-group partials and splits rows; host stacks.

Per-core dataflow (bf16 matmuls, f32 PSUM):
  stage 1: Y^T = [Q^T; K^T] tiles + V tiles from gathered xt and weights
  stage 2: S^T[k,q] = K_h Q_h^T head pairs; P^T = exp(S^T/8) on ACT
  stage 3: C~^T = [V_h|1]^T P^T (ones col -> row 64 = softmax denom);
           normalize via recip + gpsimd partition-broadcast + DVE mul
  stage 4: out_partial = C^T chunks x w_proj rows -> opart -> ReduceScatter
"""
import numpy as np
import ml_dtypes

import concourse.bacc as bacc
import concourse.tile as tile
from concourse import bass_isa, mybir
from concourse import bass_utils

P = 128
B, S, D = 4, 2048, 1024
H_TOT, HD = 16, 64
H = 8          # heads per core
NPAIR = 4      # head pairs per core
SCALE = HD ** -0.5
DCH = D // P   # 8 contraction chunks
NTT = S // P   # 16 token tiles
SH = S // 2    # tokens per I/O shard
f32 = mybir.dt.float32
bf16 = mybir.dt.bfloat16
BF = ml_dtypes.bfloat16
AF = mybir.ActivationFunctionType

_CACHED_NC = None


def build_nc():
    nc = bacc.Bacc(trn_type="TRN2", target_bir_lowering=False, debug=False,
                   num_devices=8)
    xh = nc.dram_tensor("xh", [D, SH], bf16, kind="ExternalInput").ap()
    wsh = nc.dram_tensor("wsh", [D, 512], bf16, kind="ExternalInput").ap()
    bqk = nc.dram_tensor("bqk", [8, P], f32, kind="ExternalInput").ap()
    vbrow = nc.dram_tensor("vbrow", [1, NPAIR * 130], f32,
                           kind="ExternalInput").ap()
    oh = nc.dram_tensor("oh", [SH, D], bf16, kind="ExternalOutput").ap()

    xb = nc.dram_tensor("xb", [D, SH], bf16, kind="Internal").ap()
    xg = nc.dram_tensor("xg", [2, D, SH], bf16, kind="Internal").ap()
    wb = nc.dram_tensor("wb", [D, 512], bf16, kind="Internal").ap()
    wg = nc.dram_tensor("wg", [4, D, 512], bf16, kind="Internal").ap()
    opart = nc.dram_tensor("opart", [S, D], bf16, kind="Internal").ap()
    ors = nc.dram_tensor("ors", [SH, D], bf16, kind="Internal").ap()

    with tile.TileContext(nc) as tc:
        # -------- stage 0: bounce + gather shards (weights first) --------
        nc.sync.dma_start(wb[:], wsh[:])
        nc.gpsimd.collective_compute(
            "AllGather", mybir.AluOpType.bypass,
            replica_groups=[[0, 2, 4, 6], [1, 3, 5, 7]],
            ins=[wb[:].opt()], outs=[wg[:].opt()])
        nc.scalar.dma_start(xb[:], xh[:])
        nc.gpsimd.collective_compute(
            "AllGather", mybir.AluOpType.bypass,
            replica_groups=[[0, 1], [2, 3], [4, 5], [6, 7]],
            ins=[xb[:].opt()], outs=[xg[:].opt()])

        with tc.tile_pool(name="persist", bufs=1) as pp:
            # persistent SBUF tensors
            yt = [pp.tile([P, S], bf16, name=f"yt{f}") for f in range(8)]
            vp = pp.tile([P, NTT, NPAIR * 130], bf16, name="vp")
            vbias_t = pp.tile([P, NPAIR * 130], f32, name="vbias_t")

            # vbias row replicated across partitions; ones columns of vp
            # (the last col of each 65-col half-block) set directly
            vbr = pp.tile([1, NPAIR * 130], f32, name="vbr")
            nc.gpsimd.dma_start(vbr[:], vbrow[:])
            nc.gpsimd.partition_broadcast(vbias_t[:], vbr[:])
            vp65 = vp[:].rearrange("p t (a c) -> p t a c", c=65)
            nc.vector.memset(vp65[:, :, :, 64:65], 1.0)

            # ---------------- stage 1: QKV projections ----------------
            with (
                tc.tile_pool(name="s1w", bufs=1) as s1w,
                tc.tile_pool(name="s1x", bufs=2) as s1x,
                tc.tile_pool(name="s1ps", bufs=4, space="PSUM") as s1ps,
            ):
                TCH = 256
                NCH = S // TCH
                wqk_f = [s1w.tile([P, DCH, P], bf16, name=f"wqkf{f}")
                         for f in range(8)]
                wv_t = s1w.tile([P, DCH, H * HD], bf16, name="wv_t")
                bqk_t = s1w.tile([P, 8], f32, name="bqk_t")
                # weight tiles on the ACT HWDGE ring so they don't queue
                # behind the xt stream
                for f in range(8):
                    blk = wg[f // 4].rearrange("(c p) g -> p c g", p=P)
                    col = (f % 4) * P
                    nc.scalar.dma_start(wqk_f[f][:], blk[:, :, col:col + P])
                nc.scalar.dma_start(bqk_t[:], bqk.rearrange("a p -> p a"))
                nc.scalar.dma_start(
                    wv_t[:], wg[2].rearrange("(c p) g -> p c g", p=P))

                for t in range(NCH):  # 256-token chunks
                    tsl = slice(t * TCH, (t + 1) * TCH)
                    xt_t = s1x.tile([P, DCH, TCH], bf16, name="xt_t")
                    lo = (t % 4) * TCH
                    nc.sync.dma_start(
                        xt_t[:],
                        xg[t // 4, :, lo:lo + TCH].rearrange(
                            "(c p) s -> p c s", p=P))
                    for f in range(8):  # Q,K feature tiles
                        ps = s1ps.tile([P, TCH], f32, name="s1pq")
                        for i in range(DCH):
                            nc.tensor.matmul(
                                ps[:], wqk_f[f][:, i, :], xt_t[:, i, :],
                                start=(i == 0), stop=(i == DCH - 1))
                        nc.vector.tensor_scalar(
                            out=yt[f][:, tsl], in0=ps[:],
                            scalar1=bqk_t[:, f:f + 1],
                            scalar2=None, op0=mybir.AluOpType.add)
                    for sub in range(TCH // P):  # V for 128-token subtiles
                        tt = t * (TCH // P) + sub
                        ps = s1ps.tile([P, 512], f32, name="s1p")
                        for i in range(DCH):
                            nc.tensor.matmul(
                                ps[:], xt_t[:, i, sub * P:(sub + 1) * P],
                                wv_t[:, i, :],
                                start=(i == 0), stop=(i == DCH - 1))
                        vpt = vp[:, tt, :].rearrange("p (j k c) -> p j k c",
                                                     j=NPAIR, k=2)
                        vb4 = vbias_t[:].rearrange("p (j k c) -> p j k c",
                                                   j=NPAIR, k=2)
                        nc.vector.tensor_tensor(
                            out=vpt[:, :, :, 0:HD],
                            in0=ps[:].rearrange("p (j k c) -> p j k c",
                                                j=NPAIR, k=2),
                            in1=vb4[:, :, :, 0:HD],
                            op=mybir.AluOpType.add)

            # ---------------- stages 2+3: attention ----------------
            # Per k-chunk: one [128,1024] PSUM tile holds S^T for both heads
            # of the pair (even in cols 0:512, odd in 512:1024), one ACT exp
            # per k-chunk, PV software-pipelined one k-chunk behind.
            ct = [pp.tile([P, S], bf16, name=f"ct{j}") for j in range(NPAIR)]
            with (
                tc.tile_pool(name="s4w", bufs=1) as s4w,
                tc.tile_pool(name="att", bufs=1) as att,
                tc.tile_pool(name="s4o", bufs=2) as s4o,
                tc.tile_pool(name="spt", bufs=2, space="PSUM") as sptp,
                tc.tile_pool(name="cps", bufs=2, space="PSUM") as cpsp,
                tc.tile_pool(name="s4ps", bufs=2, space="PSUM") as s4ps,
            ):
                wp_t = s4w.tile([P, NPAIR, D], bf16, name="wp_t")
                for h in range(2):
                    nc.scalar.dma_start(
                        wp_t[:, :, h * 512:(h + 1) * 512],
                        wg[3][h * 512:(h + 1) * 512, :].rearrange(
                            "(c p) f -> p c f", p=P))

                # zeros rows 0:63 + per-norm recip row 64; partition
                # all-reduce (add) replicates the recip row across partitions
                zt = att.tile([65, 1024], f32, name="zt", bufs=1)
                nc.vector.memset(zt[0:HD, :], 0.0)

                def emit_norm(j, qa, cps_e, cps_o):
                    nc.vector.reciprocal(zt[64:65, 0:512], cps_e[64:65, :])
                    nc.vector.reciprocal(zt[64:65, 512:1024], cps_o[64:65, :])
                    rbc = att.tile([65, 1024], f32, name="rbc", bufs=2)
                    nc.gpsimd.partition_all_reduce(
                        rbc[:], zt[:], channels=65,
                        reduce_op=bass_isa.ReduceOp.add)
                    nc.vector.tensor_mul(ct[j][0:HD, qa], cps_e[0:HD, :],
                                         rbc[0:HD, 0:512])
                    cttmp = att.tile([HD, 512], bf16, name="cttmp", bufs=1)
                    nc.vector.tensor_mul(cttmp[:], cps_o[0:HD, :],
                                         rbc[0:HD, 512:1024])
                    nc.sync.dma_start(ct[j][HD:P, qa], cttmp[:])

                # projection work for one token tile, emitted as a list of
                # closures so matmuls drip into the PE stream without bursts
                def proj_steps(tt):
                    tsl = slice(tt * P, (tt + 1) * P)
                    steps = []
                    state = {}

                    def mk_mm(half, fc):
                        def f():
                            if fc == 0:
                                state[half] = s4ps.tile([P, 512], f32,
                                                        name="s4p")
                            nc.tensor.matmul(
                                state[half][:], ct[fc][:, tsl],
                                wp_t[:, fc, half * 512:(half + 1) * 512],
                                start=(fc == 0), stop=(fc == NPAIR - 1))
                            if fc == NPAIR - 1:
                                o_sb = s4o.tile([P, 512], bf16, name="o_sb",
                                                bufs=4)
                                nc.vector.tensor_copy(o_sb[:], state[half][:])
                                nc.sync.dma_start(
                                    opart[tsl, half * 512:(half + 1) * 512],
                                    o_sb[:])
                        return f

                    for half in range(2):
                        for fc in range(NPAIR):
                            steps.append(mk_mm(half, fc))
                    return steps

                norm_pending = None
                proj_queue = []
                for qc in range(4):  # 512-wide query chunks, outer
                    qa = slice(qc * 512, (qc + 1) * 512)
                    for j in range(NPAIR):
                        qt, kt = yt[j], yt[NPAIR + j]
                        cps_e = cps_o = None
                        pv_pending = None
                        for kc in range(NTT):
                            ksl = slice(kc * P, (kc + 1) * P)
                            spt = sptp.tile([P, 1024], f32, name="spt")
                            nc.tensor.matmul(spt[:, 0:512], kt[0:HD, ksl],
                                             qt[0:HD, qa], start=True,
                                             stop=True)
                            nc.tensor.matmul(spt[:, 512:1024], kt[HD:P, ksl],
                                             qt[HD:P, qa], start=True,
                                             stop=True)
                            ppt = att.tile([P, 1024], bf16, name="ppt",
                                           bufs=4)
                            nc.scalar.activation(ppt[:], spt[:], AF.Exp,
                                                 scale=SCALE)
                            if kc == 1 and norm_pending is not None:
                                # previous (qc,j) normalization, deferred past
                                # this iteration's first two S/exp to hide its
                                # recip -> all-reduce -> mul chain
                                emit_norm(*norm_pending)
                                norm_pending = None
                            if pv_pending is not None:
                                if cps_e is None:
                                    cps_e = cpsp.tile([65, 512], f32,
                                                      name="cps")
                                    cps_o = cpsp.tile([65, 512], f32,
                                                      name="cps")
                                _emit_pv(nc, cps_e, cps_o, vp, pv_pending[0],
                                         pv_pending[1], j)
                            pv_pending = (kc, ppt)
                            if proj_queue and kc % 2 == 1:
                                proj_queue.pop(0)()  # drip one proj step
                        _emit_pv(nc, cps_e, cps_o, vp, pv_pending[0],
                                 pv_pending[1], j)
                        norm_pending = (j, qa, cps_e, cps_o)
                    # queue projection for this query chunk's 4 token tiles
                    # (runnable once this qc's last norm flushes next sweep)
                    for tt in range(qc * 4, (qc + 1) * 4):
                        proj_queue.extend(proj_steps(tt))
                emit_norm(*norm_pending)
                for step in proj_queue:
                    step()

        # -------- stage 5: sum head-group partials, emit own row half ----
        nc.gpsimd.collective_compute(
            "ReduceScatter", mybir.AluOpType.add,
            replica_groups=[[0, 1], [2, 3], [4, 5], [6, 7]],
            ins=[opart[:].opt()], outs=[ors[:].opt()])
        nc.sync.dma_start(oh[:], ors[:])

    nc.finalize()
    return nc


def _emit_pv(nc, cps_e, cps_o, vp, kc, ppt, j):
    nc.tensor.matmul(cps_e[0:65, :], vp[:, kc, j * 130:j * 130 + 65],
                     ppt[:, 0:512], start=(kc == 0), stop=(kc == NTT - 1))
    nc.tensor.matmul(cps_o[0:65, :], vp[:, kc, j * 130 + 65:j * 130 + 130],
                     ppt[:, 512:1024], start=(kc == 0), stop=(kc == NTT - 1))


def get_nc():
    global _CACHED_NC
    if _CACHED_NC is None:
        _CACHED_NC = build_nc()
    return _CACHED_NC


def make_in_maps(x, w_qkv, b_qkv, w_proj):
    """Host-side sharding: disjoint bf16 shards, one dict per core."""
    x = np.asarray(x, np.float32)
    w3 = np.asarray(w_qkv, np.float32).reshape(D, 3, H_TOT, HD)
    bq3 = np.asarray(b_qkv, np.float32).reshape(3, H_TOT, HD)
    wp_full = np.asarray(w_proj, np.float32).reshape(H_TOT, HD, D)

    wgrp, bqk_g, vbrow_g = [], [], []
    for hg in range(2):
        hs = slice(hg * H, (hg + 1) * H)
        wqk_g = np.concatenate([w3[:, 0, hs].reshape(D, H * HD),
                                w3[:, 1, hs].reshape(D, H * HD)], axis=1)
        wv_g = w3[:, 2, hs].reshape(D, H * HD)
        wp_g = wp_full[hs].reshape(H * HD, D)
        wp_pack = np.concatenate([wp_g[:, 0:512], wp_g[:, 512:1024]], axis=0)
        wgrp.append(np.ascontiguousarray(
            np.concatenate([wqk_g, wv_g, wp_pack], axis=1)).astype(BF))
        bqk_g.append(np.ascontiguousarray(
            np.concatenate([bq3[0, hs].reshape(H * HD),
                            bq3[1, hs].reshape(H * HD)]).reshape(8, P)))
        bv = bq3[2, hs].reshape(H * HD)
        vb = np.zeros((1, NPAIR * 130), np.float32)
        for j in range(NPAIR):
            vb[0, j * 130:j * 130 + HD] = bv[(2 * j) * HD:(2 * j + 1) * HD]
            vb[0, j * 130 + 65:j * 130 + 65 + HD] = \
                bv[(2 * j + 1) * HD:(2 * j + 2) * HD]
        vbrow_g.append(vb)

    in_maps = []
    for c in range(8):
        b, hg, k = c // 2, c % 2, c // 2
        xt_c = np.ascontiguousarray(
            x[b].T[:, SH * hg:SH * (hg + 1)]).astype(BF)
        wsh_c = np.ascontiguousarray(wgrp[hg][:, 512 * k:512 * (k + 1)])
        in_maps.append({"xh": xt_c, "wsh": wsh_c, "bqk": bqk_g[hg],
                        "vbrow": vbrow_g[hg]})
    return in_maps


def assemble(results, b_proj):
    out = np.empty((B, S, D), np.float32)
    bp = np.asarray(b_proj, np.float32)
    for b in range(B):
        out[b, 0:SH] = results[2 * b]["oh"].astype(np.float32)
        out[b, SH:S] = results[2 * b + 1]["oh"].astype(np.float32)
        out[b] += bp
    return out


def kernel(x, w_qkv, b_qkv, w_proj, b_proj):
    nc = get_nc()
    in_maps = make_in_maps(x, w_qkv, b_qkv, w_proj)
    res = bass_utils.run_bass_kernel_spmd(nc, in_maps, core_ids=list(range(8)),
                                          trace=False)
    return assemble(res.results, b_proj)


# revision 13
# speedup vs baseline: 11.8908x; 2.8456x over previous
"""Multi-head self-attention (B=4, S=2048, D=1024, H=16) on 8 TRN2 NeuronCores.

Host I/O is the bottleneck for this problem (per-call staging of declared
I/O buffers costs ~78us/MB in + ~94us/MB out, dwarfing the ~0.5ms compute),
so the design minimizes declared bytes: every input byte is declared on
exactly one core, in bf16, and redistributed on-device with collectives.

Sharding: batch x head-group compute (core c: batch b=c//2, heads
[8*(c%2), 8*(c%2)+8)), but disjoint I/O shards:
  xha/xhb [D, 512] bf16 -- x[b]^T token-half c%2, in two pieces
                           (pair-AllGather each -> full xt)
  wshqv [D, 384], wshp [D, 128] bf16 -- quarter of this head-group's packed
                           wqk+wv / wp (4-core AllGather over {c%2, c%2+2..})
  oh [4, 256, D] bf16 -- per-query-chunk ReduceScatter(add) over the batch
                           pair sums the two head-group partials; host maps
                           the row quarters back.

Collective order xa, wqkv, xb, wp pipelines the gathers against stage-1
compute; the per-chunk ReduceScatters overlap the output exchange with the
projection drip of the following sweep.

Per-core dataflow (bf16 matmuls, f32 PSUM):
  stage 1: Y^T = [Q^T; K^T] tiles + V tiles from gathered xt and weights
  stage 2: S^T[k,q] = K_h Q_h^T head pairs; P^T = exp(S^T/8) on ACT
  stage 3: C~^T = [V_h|1]^T P^T (ones col -> row 64 = softmax denom);
           normalize via recip + gpsimd partition-broadcast + DVE mul
  stage 4: out_partial = C^T chunks x w_proj rows -> opart -> ReduceScatter
"""
import numpy as np
import ml_dtypes

import concourse.bacc as bacc
import concourse.tile as tile
from concourse import bass_isa, mybir
from concourse import bass_utils

P = 128
B, S, D = 4, 2048, 1024
H_TOT, HD = 16, 64
H = 8          # heads per core
NPAIR = 4      # head pairs per core
SCALE = HD ** -0.5
DCH = D // P   # 8 contraction chunks
NTT = S // P   # 16 token tiles
SH = S // 2    # tokens per I/O shard
f32 = mybir.dt.float32
bf16 = mybir.dt.bfloat16
BF = ml_dtypes.bfloat16
AF = mybir.ActivationFunctionType

G4 = [[0, 2, 4, 6], [1, 3, 5, 7]]   # head-group weight replica groups
G2 = [[0, 1], [2, 3], [4, 5], [6, 7]]  # batch-pair replica groups

_CACHED_NC = None


def build_nc():
    nc = bacc.Bacc(trn_type="TRN2", target_bir_lowering=False, debug=False,
                   num_devices=8)
    xha = nc.dram_tensor("xha", [D, 512], bf16, kind="ExternalInput").ap()
    xhb = nc.dram_tensor("xhb", [D, 512], bf16, kind="ExternalInput").ap()
    wshqk = nc.dram_tensor("wshqk", [D, 256], bf16, kind="ExternalInput").ap()
    wshv = nc.dram_tensor("wshv", [D, 128], bf16, kind="ExternalInput").ap()
    wshp = nc.dram_tensor("wshp", [D, 128], bf16, kind="ExternalInput").ap()
    bqk = nc.dram_tensor("bqk", [8, P], f32, kind="ExternalInput").ap()
    vbrow = nc.dram_tensor("vbrow", [1, NPAIR * 130], f32,
                           kind="ExternalInput").ap()
    oh = nc.dram_tensor("oh", [4, 256, D], bf16, kind="ExternalOutput").ap()

    xba = nc.dram_tensor("xba", [D, 512], bf16, kind="Internal").ap()
    xbb = nc.dram_tensor("xbb", [D, 512], bf16, kind="Internal").ap()
    xga = nc.dram_tensor("xga", [2, D, 512], bf16, kind="Internal").ap()
    xgb = nc.dram_tensor("xgb", [2, D, 512], bf16, kind="Internal").ap()
    wbqk = nc.dram_tensor("wbqk", [D, 256], bf16, kind="Internal").ap()
    wbv = nc.dram_tensor("wbv", [D, 128], bf16, kind="Internal").ap()
    wbp = nc.dram_tensor("wbp", [D, 128], bf16, kind="Internal").ap()
    wgqk = nc.dram_tensor("wgqk", [4, D, 256], bf16, kind="Internal").ap()
    wgv = nc.dram_tensor("wgv", [4, D, 128], bf16, kind="Internal").ap()
    wgp = nc.dram_tensor("wgp", [4, D, 128], bf16, kind="Internal").ap()
    opart = nc.dram_tensor("opart", [S, D], bf16, kind="Internal").ap()
    ors = nc.dram_tensor("ors", [4, 256, D], bf16, kind="Internal").ap()

    def ag(groups, in_ap, out_ap):
        nc.gpsimd.collective_compute(
            "AllGather", mybir.AluOpType.bypass, replica_groups=groups,
            ins=[in_ap.opt()], outs=[out_ap.opt()])

    with tile.TileContext(nc) as tc:
        # ---- stage 0: bounce + pipelined shard gathers ----
        # all bounces on ONE ring, in desired collective order: the collective
        # queue drains FIFO by input-readiness, so bounce completion order
        # must equal the wanted gather order (wqk, xa, xb, wv, wp); wv/wp are
        # not needed until PV / the projection drip, so they gather last
        nc.sync.dma_start(wbqk[:], wshqk[:])
        nc.sync.dma_start(xba[:], xha[:])
        nc.sync.dma_start(xbb[:], xhb[:])
        nc.sync.dma_start(wbv[:], wshv[:])
        nc.sync.dma_start(wbp[:], wshp[:])
        ag(G4, wbqk[:], wgqk[:])
        ag(G2, xba[:], xga[:])
        ag(G2, xbb[:], xgb[:])
        ag(G4, wbv[:], wgv[:])
        ag(G4, wbp[:], wgp[:])

        with tc.tile_pool(name="persist", bufs=1) as pp:
            # persistent SBUF tensors
            yt = [pp.tile([P, S], bf16, name=f"yt{f}") for f in range(8)]
            vp = pp.tile([P, NTT, NPAIR * 130], bf16, name="vp")
            vbias_t = pp.tile([P, NPAIR * 130], f32, name="vbias_t")

            # vbias row replicated across partitions; ones columns of vp
            # (the last col of each 65-col half-block) set directly
            vbr = pp.tile([1, NPAIR * 130], f32, name="vbr")
            nc.gpsimd.dma_start(vbr[:], vbrow[:])
            nc.gpsimd.partition_broadcast(vbias_t[:], vbr[:])
            vp65 = vp[:].rearrange("p t (a c) -> p t a c", c=65)
            nc.vector.memset(vp65[:, :, :, 64:65], 1.0)

            # ---------------- stage 1: QK projections ----------------
            with (
                tc.tile_pool(name="s1w", bufs=1) as s1w,
                tc.tile_pool(name="s1x", bufs=8) as s1x,
            ):
                TCH = 256
                wqk_f = [s1w.tile([P, DCH, P], bf16, name=f"wqkf{f}")
                         for f in range(8)]
                wv_t = s1w.tile([P, DCH, H * HD], bf16, name="wv_t")
                bqk_t = s1w.tile([P, 8], f32, name="bqk_t")
                # weight tiles on the ACT HWDGE ring so they don't queue
                # behind the xt stream
                for f in range(8):
                    blk = wgqk[f // 2].rearrange("(c p) g -> p c g", p=P)
                    col = (f % 2) * P
                    nc.scalar.dma_start(wqk_f[f][:], blk[:, :, col:col + P])
                nc.scalar.dma_start(bqk_t[:], bqk.rearrange("a p -> p a"))
                for k in range(4):
                    nc.scalar.dma_start(
                        wv_t[:, :, P * k:P * (k + 1)],
                        wgv[k].rearrange("(c p) g -> p c g", p=P))

                xt_ts = {}
                # 256-token chunks in xga-first order so compute starts
                # before the second x gather lands; V is NOT done here -- its
                # matmuls fold into the first attention sweep so they never
                # stall QK or the exp stream behind the late wv gather
                with tc.tile_pool(name="s1ps", bufs=4, space="PSUM") as s1ps:
                    for t in (0, 1, 4, 5, 2, 3, 6, 7):
                        tsl = slice(t * TCH, (t + 1) * TCH)
                        h, r = t // 4, (t * TCH) % 1024
                        xg, lo = (xga, r) if r < 512 else (xgb, r - 512)
                        xt_t = s1x.tile([P, DCH, TCH], bf16, name="xt_t")
                        xt_ts[t] = xt_t
                        nc.sync.dma_start(
                            xt_t[:],
                            xg[h, :, lo:lo + TCH].rearrange(
                                "(c p) s -> p c s", p=P))
                        for f in range(8):  # Q,K feature tiles
                            ps = s1ps.tile([P, TCH], f32, name="s1pq")
                            for i in range(DCH):
                                nc.tensor.matmul(
                                    ps[:], wqk_f[f][:, i, :], xt_t[:, i, :],
                                    start=(i == 0), stop=(i == DCH - 1))
                            nc.vector.tensor_scalar(
                                out=yt[f][:, tsl], in0=ps[:],
                                scalar1=bqk_t[:, f:f + 1],
                                scalar2=None, op0=mybir.AluOpType.add)

                def emit_v(tt):  # V for one 128-token subtile, into vp[:, tt]
                    ps = flex.tile([P, 512], f32, name="fxp")
                    xt_t = xt_ts[tt // 2]
                    sub = tt % 2
                    for i in range(DCH):
                        nc.tensor.matmul(
                            ps[:], xt_t[:, i, sub * P:(sub + 1) * P],
                            wv_t[:, i, :],
                            start=(i == 0), stop=(i == DCH - 1))
                    vpt = vp[:, tt, :].rearrange("p (j k c) -> p j k c",
                                                 j=NPAIR, k=2)
                    vb4 = vbias_t[:].rearrange("p (j k c) -> p j k c",
                                               j=NPAIR, k=2)
                    nc.vector.tensor_tensor(
                        out=vpt[:, :, :, 0:HD],
                        in0=ps[:].rearrange("p (j k c) -> p j k c",
                                            j=NPAIR, k=2),
                        in1=vb4[:, :, :, 0:HD],
                        op=mybir.AluOpType.add)

                # ---------------- stages 2+3: attention ----------------
                # Per k-chunk: one [128,1024] PSUM tile holds S^T for both
                # heads of the pair (even in cols 0:512, odd in 512:1024),
                # one ACT exp per k-chunk, PV software-pipelined one k-chunk
                # behind.
                ct = [pp.tile([P, S], bf16, name=f"ct{j}")
                      for j in range(NPAIR)]
                with (
                    tc.tile_pool(name="s4w", bufs=1) as s4w,
                    tc.tile_pool(name="att", bufs=1) as att,
                    tc.tile_pool(name="s4o", bufs=2) as s4o,
                    tc.tile_pool(name="spt", bufs=2, space="PSUM") as sptp,
                    tc.tile_pool(name="cps", bufs=2, space="PSUM") as cpsp,
                    tc.tile_pool(name="flex", bufs=2, space="PSUM") as flex,
                ):
                    wp_t = s4w.tile([P, NPAIR, D], bf16, name="wp_t")
                    # on the sync ring: the ACT ring would head-of-line
                    # block the exp stream behind the late wp gather
                    for k in range(4):
                        for h in range(2):
                            nc.sync.dma_start(
                                wp_t[:, :,
                                     h * 512 + P * k:h * 512 + P * (k + 1)],
                                wgp[k][h * 512:(h + 1) * 512, :].rearrange(
                                    "(c p) f -> p c f", p=P))

                    # zeros rows 0:63 + per-norm recip row 64; partition
                    # all-reduce (add) replicates the recip row across
                    # partitions
                    zt = att.tile([65, 1024], f32, name="zt", bufs=1)
                    nc.vector.memset(zt[0:HD, :], 0.0)

                    def emit_norm(j, qa, cps_e, cps_o):
                        nc.vector.reciprocal(zt[64:65, 0:512],
                                             cps_e[64:65, :])
                        nc.vector.reciprocal(zt[64:65, 512:1024],
                                             cps_o[64:65, :])
                        rbc = att.tile([65, 1024], f32, name="rbc", bufs=2)
                        nc.gpsimd.partition_all_reduce(
                            rbc[:], zt[:], channels=65,
                            reduce_op=bass_isa.ReduceOp.add)
                        nc.vector.tensor_mul(ct[j][0:HD, qa], cps_e[0:HD, :],
                                             rbc[0:HD, 0:512])
                        cttmp = att.tile([HD, 512], bf16, name="cttmp",
                                         bufs=1)
                        nc.vector.tensor_mul(cttmp[:], cps_o[0:HD, :],
                                             rbc[0:HD, 512:1024])
                        nc.sync.dma_start(ct[j][HD:P, qa], cttmp[:])

                    def emit_rs(q):
                        nc.gpsimd.collective_compute(
                            "ReduceScatter", mybir.AluOpType.add,
                            replica_groups=G2,
                            ins=[opart[512 * q:512 * (q + 1), :].opt()],
                            outs=[ors[q].opt()])
                        nc.sync.dma_start(oh[q], ors[q])

                    # projection work for one token tile, emitted as a list
                    # of closures so matmuls drip into the PE stream without
                    # bursts
                    def proj_steps(tt):
                        tsl = slice(tt * P, (tt + 1) * P)
                        steps = []
                        state = {}

                        def mk_mm(half, fc):
                            def f():
                                if fc == 0:
                                    state[half] = flex.tile([P, 512], f32,
                                                            name="fxp")
                                nc.tensor.matmul(
                                    state[half][:], ct[fc][:, tsl],
                                    wp_t[:, fc, half * 512:(half + 1) * 512],
                                    start=(fc == 0), stop=(fc == NPAIR - 1))
                                if fc == NPAIR - 1:
                                    o_sb = s4o.tile([P, 512], bf16,
                                                    name="o_sb", bufs=4)
                                    nc.vector.tensor_copy(o_sb[:],
                                                          state[half][:])
                                    nc.sync.dma_start(
                                        opart[tsl,
                                              half * 512:(half + 1) * 512],
                                        o_sb[:])
                            return f

                        for half in range(2):
                            for fc in range(NPAIR):
                                steps.append(mk_mm(half, fc))
                        return steps

                    norm_pending = None
                    proj_queue = []  # (step, rs_q_after or None)
                    for qc in range(4):  # 512-wide query chunks, outer
                        qa = slice(qc * 512, (qc + 1) * 512)
                        for j in range(NPAIR):
                            qt, kt = yt[j], yt[NPAIR + j]
                            cps_e = cps_o = None
                            pv_pending = None
                            for kc in range(NTT):
                                ksl = slice(kc * P, (kc + 1) * P)
                                spt = sptp.tile([P, 1024], f32, name="spt")
                                nc.tensor.matmul(spt[:, 0:512], kt[0:HD, ksl],
                                                 qt[0:HD, qa], start=True,
                                                 stop=True)
                                nc.tensor.matmul(spt[:, 512:1024],
                                                 kt[HD:P, ksl],
                                                 qt[HD:P, qa], start=True,
                                                 stop=True)
                                if qc == 0 and j == 0:
                                    emit_v(kc)  # V folded into first sweep
                                ppt = att.tile([P, 1024], bf16, name="ppt",
                                               bufs=4)
                                nc.scalar.activation(ppt[:], spt[:], AF.Exp,
                                                     scale=SCALE)
                                if kc == 1 and norm_pending is not None:
                                    # previous (qc,j) normalization, deferred
                                    # past this iteration's first two S/exp to
                                    # hide its recip -> all-reduce -> mul
                                    # chain
                                    emit_norm(*norm_pending)
                                    norm_pending = None
                                if pv_pending is not None:
                                    if cps_e is None:
                                        cps_e = cpsp.tile([65, 512], f32,
                                                          name="cps")
                                        cps_o = cpsp.tile([65, 512], f32,
                                                          name="cps")
                                    _emit_pv(nc, cps_e, cps_o, vp,
                                             pv_pending[0], pv_pending[1], j)
                                pv_pending = (kc, ppt)
                                if proj_queue and kc % 2 == 1:
                                    step, rs_q = proj_queue.pop(0)
                                    step()  # drip one proj step
                                    if rs_q is not None:
                                        emit_rs(rs_q)
                            _emit_pv(nc, cps_e, cps_o, vp, pv_pending[0],
                                     pv_pending[1], j)
                            norm_pending = (j, qa, cps_e, cps_o)
                        # queue projection for this query chunk's 4 token
                        # tiles (runnable once this qc's last norm flushes
                        # next sweep); the matching ReduceScatter follows the
                        # final step
                        steps = []
                        for tt in range(qc * 4, (qc + 1) * 4):
                            steps.extend(proj_steps(tt))
                        proj_queue.extend(
                            (s, qc if i == len(steps) - 1 else None)
                            for i, s in enumerate(steps))
                    emit_norm(*norm_pending)
                    for step, rs_q in proj_queue:
                        step()
                        if rs_q is not None:
                            emit_rs(rs_q)

    nc.finalize()
    return nc


def _emit_pv(nc, cps_e, cps_o, vp, kc, ppt, j):
    nc.tensor.matmul(cps_e[0:65, :], vp[:, kc, j * 130:j * 130 + 65],
                     ppt[:, 0:512], start=(kc == 0), stop=(kc == NTT - 1))
    nc.tensor.matmul(cps_o[0:65, :], vp[:, kc, j * 130 + 65:j * 130 + 130],
                     ppt[:, 512:1024], start=(kc == 0), stop=(kc == NTT - 1))


def get_nc():
    global _CACHED_NC
    if _CACHED_NC is None:
        _CACHED_NC = build_nc()
    return _CACHED_NC


def make_in_maps(x, w_qkv, b_qkv, w_proj):
    """Host-side sharding: disjoint bf16 shards, one dict per core."""
    x = np.asarray(x, np.float32)
    w3 = np.asarray(w_qkv, np.float32).reshape(D, 3, H_TOT, HD)
    bq3 = np.asarray(b_qkv, np.float32).reshape(3, H_TOT, HD)
    wp_full = np.asarray(w_proj, np.float32).reshape(H_TOT, HD, D)

    wqk_g, wv_g, wp_g, bqk_g, vbrow_g = [], [], [], [], []
    for hg in range(2):
        hs = slice(hg * H, (hg + 1) * H)
        wqk_g.append(np.concatenate(
            [w3[:, 0, hs].reshape(D, H * HD),
             w3[:, 1, hs].reshape(D, H * HD)], axis=1).astype(BF))
        wv_g.append(w3[:, 2, hs].reshape(D, H * HD).astype(BF))
        wpg = wp_full[hs].reshape(H * HD, D)
        wp_g.append(np.concatenate(
            [wpg[:, 0:512], wpg[:, 512:1024]], axis=0).astype(BF))
        bqk_g.append(np.ascontiguousarray(
            np.concatenate([bq3[0, hs].reshape(H * HD),
                            bq3[1, hs].reshape(H * HD)]).reshape(8, P)))
        bv = bq3[2, hs].reshape(H * HD)
        vb = np.zeros((1, NPAIR * 130), np.float32)
        for j in range(NPAIR):
            vb[0, j * 130:j * 130 + HD] = bv[(2 * j) * HD:(2 * j + 1) * HD]
            vb[0, j * 130 + 65:j * 130 + 65 + HD] = \
                bv[(2 * j + 1) * HD:(2 * j + 2) * HD]
        vbrow_g.append(vb)

    in_maps = []
    for c in range(8):
        b, hg, k = c // 2, c % 2, c // 2
        xt_c = x[b].T[:, SH * hg:SH * (hg + 1)].astype(BF)
        in_maps.append({
            "xha": np.ascontiguousarray(xt_c[:, 0:512]),
            "xhb": np.ascontiguousarray(xt_c[:, 512:1024]),
            "wshqk": np.ascontiguousarray(wqk_g[hg][:, 256 * k:256 * (k + 1)]),
            "wshv": np.ascontiguousarray(wv_g[hg][:, P * k:P * (k + 1)]),
            "wshp": np.ascontiguousarray(wp_g[hg][:, P * k:P * (k + 1)]),
            "bqk": bqk_g[hg], "vbrow": vbrow_g[hg]})
    return in_maps


def assemble(results, b_proj):
    out = np.empty((B, S, D), np.float32)
    bp = np.asarray(b_proj, np.float32)
    for b in range(B):
        for th in range(2):
            oh = results[2 * b + th]["oh"].astype(np.float32)
            for q in range(4):
                out[b, 512 * q + 256 * th:512 * q + 256 * (th + 1)] = oh[q]
        out[b] += bp
    return out


def kernel(x, w_qkv, b_qkv, w_proj, b_proj):
    nc = get_nc()
    in_maps = make_in_maps(x, w_qkv, b_qkv, w_proj)
    res = bass_utils.run_bass_kernel_spmd(nc, in_maps, core_ids=list(range(8)),
                                          trace=False)
    return assemble(res.results, b_proj)
